# revision 27
# baseline (speedup 1.0000x reference)
"""Trainium2 Bass kernel for nn_ClassLayer_56564719289025.

Reference computation:  y = mean(|W|) * (x @ sign(W).T)
  x: [8192, 4096] f32, W: [4096, 4096] f32 -> y: [8192, 4096] f32

Strategy (8 NeuronCores):
  - Data-parallel over x rows: each core computes a 1024-row shard of y.
  - W is replicated; sign(W) is exactly representable in bf16, so the
    matmul runs in bf16 (x rounded to bf16; error ~1e-3 relative).
  - scale = mean(|W|) is computed on-device: each core abs-sums a
    distinct 512-column slice of W^T (1/8 of W), then a 512-byte
    AllReduce across the 8 cores combines the partials.
  - Host-side prep is layout-only: bf16 cast + transposes + sharding
    (the contraction dim must be the SBUF partition dim on both matmul
    operands, so both x and W are shipped transposed).

Per-core loop: x^T shard resident in SBUF (bf16, 8.4MB); W^T streamed
in 8 o-blocks of 512 columns; ACT engine converts W tiles to sign tiles;
PE runs 2048 accumulating [128x128]@[128x512] bf16 matmuls; DVE evicts
PSUM with a fused multiply by the broadcast scale.
"""

import numpy as np
import ml_dtypes

import concourse.bacc as bacc
import concourse.bass_isa as bass_isa
import concourse.mybir as mybir
import concourse.tile as tile
from concourse.bass_utils import run_bass_kernel_spmd

TOKENS, D_IN, D_OUT, N_CORES = 8192, 4096, 4096, 8
P = 128            # SBUF partitions / matmul contraction tile
OB = 512           # output-column block (one PSUM bank at fp32)
R_SHARD = TOKENS // N_CORES   # 1024 rows per core
KO = D_IN // P                # 32 contraction tiles
NB = D_OUT // OB              # 8 o-blocks
RT = R_SHARD // P             # 8 row tiles per core
SCAN_W = D_OUT // N_CORES     # 512-column scan slice per core
INV_N = 1.0 / (D_IN * D_OUT)  # exactly 2**-24

bf16 = mybir.dt.bfloat16
fp32 = mybir.dt.float32


def _emit(tc, xT, wT, wS, y, part, red, reps=1):
    nc = tc.nc
    xT3 = xT.rearrange("(ko p) r -> p ko r", p=P)   # [128, 32, 1024]
    wT3 = wT.rearrange("(ko p) o -> p ko o", p=P)   # [128, 32, 4096]
    wS3 = wS.rearrange("(ko p) o -> p ko o", p=P)   # [128, 32, 512]
    y3 = y.rearrange("(rt p) o -> p rt o", p=P)     # [128, 8, 4096]

    with (
        tc.tile_pool(name="xpool", bufs=1) as xpool,
        tc.tile_pool(name="spool", bufs=2) as spool,
        tc.tile_pool(name="wmain", bufs=8) as wmain,
        tc.tile_pool(name="wscan", bufs=2) as wscan,
        tc.tile_pool(name="scpool", bufs=1) as scpool,
        tc.tile_pool(name="ypool", bufs=12) as ypool,
        tc.tile_pool(name="psum", bufs=8, space="PSUM") as psum,
    ):
        for _ in range(reps):
            def produce_S(b):
                S_b = spool.tile([P, KO, OB], bf16, tag="S")
                for k in range(KO):
                    w_t = wmain.tile([P, OB], bf16, tag="w")
                    nc.sync.dma_start(w_t[:], wT3[:, k, b * OB:(b + 1) * OB])
                    nc.scalar.sign(S_b[:, k, :], w_t[:])
                return S_b

            # --- x and block-0 W loaded k-sliced, interleaved so the k-outer
            # block-0 matmuls can start after just one (x[k], S0[k]) pair and
            # consume W at 1 tile per 8 matmuls (well under DMA rate) ---
            x_sb = xpool.tile([P, KO, R_SHARD], bf16, tag="x")
            S0 = spool.tile([P, KO, OB], bf16, tag="S")
            for k in range(KO):
                nc.sync.dma_start(x_sb[:, k, :], xT3[:, k, :])
                w_t = wmain.tile([P, OB], bf16, tag="w")
                nc.sync.dma_start(w_t[:], wT3[:, k, 0:OB])
                nc.scalar.sign(S0[:, k, :], w_t[:])

            acc = scpool.tile([P, KO], fp32, tag="acc")

            def scan_j(j):
                ws_t = wscan.tile([P, 8, OB], bf16, tag="ws")
                nc.sync.dma_start(ws_t[:], wS3[:, j * 8:(j + 1) * 8, :])
                nc.vector.tensor_reduce(
                    acc[:, j * 8:(j + 1) * 8], ws_t[:],
                    axis=mybir.AxisListType.X, op=mybir.AluOpType.add,
                    apply_absolute_value=True,
                )

            for j in range(4):
                scan_j(j)

            # --- finish scale: partition reduce, cross-core AllReduce ---
            acc1 = scpool.tile([P, 1], fp32, tag="acc1")
            nc.vector.tensor_reduce(
                acc1[:], acc[:], axis=mybir.AxisListType.X, op=mybir.AluOpType.add
            )
            accs = scpool.tile([P, 1], fp32, tag="accs")
            nc.vector.tensor_scalar_mul(accs[:], acc1[:], INV_N)
            par_t = scpool.tile([P, 1], fp32, tag="par")
            nc.gpsimd.partition_all_reduce(
                par_t[:], accs[:], channels=P, reduce_op=bass_isa.ReduceOp.add
            )
            nc.sync.dma_start(part[:], par_t[:])
            nc.gpsimd.collective_compute(
                "AllReduce", mybir.AluOpType.add,
                [list(range(N_CORES))], [part[:]], [red[:]],
            )
            scale_sb = scpool.tile([P, 1], fp32, tag="scale")
            nc.sync.dma_start(scale_sb[:], red[:])

            # --- block 1 prefetch ---
            S1 = produce_S(1)

            def evict(ps, r, b):
                # two-step eviction: DVE copy frees the PSUM bank without
                # waiting on scale; the scale multiply binds later
                y_t = ypool.tile([P, OB], fp32, tag="y")
                nc.vector.tensor_copy(out=y_t[:], in_=ps[:])
                nc.vector.tensor_scalar_mul(y_t[:], y_t[:], scale_sb[:])
                nc.sync.dma_start(y3[:, r, b * OB:(b + 1) * OB], y_t[:])

            # --- block 0: k-outer over 8 concurrent PSUM banks, so the PE
            # starts on the first (x[k], S0[k]) pair and tracks DMA supply ---
            ps0 = [
                psum.tile([P, OB], fp32, tag="ps", name=f"ps0_{r}")
                for r in range(RT)
            ]
            for k in range(KO):
                for r in range(RT):
                    nc.tensor.matmul(
                        ps0[r][:],
                        lhsT=x_sb[:, k, r * P:(r + 1) * P],
                        rhs=S0[:, k, :],
                        start=(k == 0),
                        stop=(k == KO - 1),
                    )
            for r in range(RT):
                evict(ps0[r], r, 0)

            # --- blocks 1..7: r-inner, k-accumulate per group ---
            for b in range(1, NB):
                S_b = S1 if b == 1 else produce_S(b)
                for r in range(RT):
                    ps = psum.tile([P, OB], fp32, tag="ps")
                    for k in range(KO):
                        nc.tensor.matmul(
                            ps[:],
                            lhsT=x_sb[:, k, r * P:(r + 1) * P],
                            rhs=S_b[:, k, :],
                            start=(k == 0),
                            stop=(k == KO - 1),
                        )
                    evict(ps, r, b)


def build(reps=1):
    nc = bacc.Bacc(
        "TRN2", target_bir_lowering=False, debug=False, num_devices=N_CORES
    )
    xT = nc.dram_tensor("xT", [D_IN, R_SHARD], bf16, kind="ExternalInput").ap()
    wT = nc.dram_tensor("wT", [D_IN, D_OUT], bf16, kind="ExternalInput").ap()
    wS = nc.dram_tensor("wscan", [D_IN, SCAN_W], bf16, kind="ExternalInput").ap()
    y = nc.dram_tensor("y", [R_SHARD, D_OUT], fp32, kind="ExternalOutput").ap()
    part = nc.dram_tensor("part", [P, 1], fp32, kind="Internal").ap()
    red = nc.dram_tensor("red", [P, 1], fp32, kind="Internal", addr_space="Shared").ap()

    with tile.TileContext(nc) as tc:
        _emit(tc, xT, wT, wS, y, part, red, reps=reps)
    nc.compile()
    return nc


_NC_CACHE = {}


def _get_nc(reps=1):
    if reps not in _NC_CACHE:
        _NC_CACHE[reps] = build(reps)
    return _NC_CACHE[reps]


def _make_in_maps(x, weight):
    xb = np.asarray(x).astype(ml_dtypes.bfloat16)
    wb = np.asarray(weight).astype(ml_dtypes.bfloat16)
    xTb = np.ascontiguousarray(xb.T)   # [D_IN, TOKENS]
    wTb = np.ascontiguousarray(wb.T)   # [D_IN, D_OUT]
    in_maps = []
    for c in range(N_CORES):
        in_maps.append({
            "xT": np.ascontiguousarray(xTb[:, c * R_SHARD:(c + 1) * R_SHARD]),
            "wT": wTb,
            "wscan": np.ascontiguousarray(wTb[:, c * SCAN_W:(c + 1) * SCAN_W]),
        })
    return in_maps


def kernel(x, weight):
    x = np.asarray(x)
    weight = np.asarray(weight)
    assert x.shape == (TOKENS, D_IN), x.shape
    assert weight.shape == (D_OUT, D_IN), weight.shape
    in_maps = _make_in_maps(x, weight)
    nc = _get_nc(1)
    last_exc = None
    for attempt in range(3):
        try:
            res = run_bass_kernel_spmd(nc, in_maps, core_ids=list(range(N_CORES)))
            break
        except Exception as e:  # transient NRT device errors — retry
            last_exc = e
            import time as _time

            _time.sleep(2.0 * (attempt + 1))
    else:
        raise last_exc
    return np.concatenate(
        [res.results[c]["y"] for c in range(N_CORES)], axis=0
    ).astype(np.float32)
